# revision 27
# baseline (speedup 1.0000x reference)
"""Trainium2 Bass kernel for AnchoredMultiHeadAttention (sparse_attention).

Reference computation (per batch b, Np=Ns=D=512, H=8 heads, HD=64):
  Q = P@Wq+bq ; K = S@Wk+bk ; V = S@Wv+bv
  scores = einsum('phd,shd->ps', Q, K) * (HD^-0.5 / H) + spatial_bias
  scores = where(mask==0, -1e9, scores)                  -> output 0
  probs = softmax(scores, -1)
  output = (probs @ V) @ Wo + bo
  value = output.mean(0) @ Wval + bval                   -> output 1  [B,1]
  compat = (P@Wc_p)[:,None] + (S@Wc_s)[None,:] + bc      -> output 2

Key simplifications:
  * The per-head einsum sums over BOTH h and d, so scores == (Q @ K^T) / 64
    over the full D=512 — no head splitting anywhere.
  * probs/output are never returned; only `value` (a scalar per batch)
    depends on them.  value needs only colsum_probs[s] = sum_p probs[p,s]:
        a0 = colsum_probs @ V0 + Np*bv   (V0 = S@Wv; sum_s colsum = Np)
        mean_out = (a0/Np) @ Wo + bo ;  value = mean_out . Wval + bval
    The probs matrix is reduced on the fly by [128,1]x[128,512] matmuls
    (lhsT = 1/rowsum per row-chunk); nothing big is ever transposed.
  * Masking: scores_out = raw/64 + spatial_bias + (mask-1)*1e9 which matches
    where(mask==0,-1e9,.) to ~6e-8 of the 1e9 absmax.

Engine discipline (hardware constraint): f32r matmuls are self-loading
(single S3_LW instruction) and can carry only ONE semaphore wait.  So every
matmul-operand tile is produced by a DVE instruction (funnel self-copies for
DMA'd tensors, tensor_scalar/tensor_copy for computed ones) and every PSUM
bank is released by a DVE reader — all matmul waits collapse onto the DVE
semaphore.  ACT only does exp into an f32 staging tile and the compat bias
epilogue; GPSIMD builds the additive mask bias.

Sharding: pure data-parallel — batch b -> core b (B=8), weights replicated,
no collectives.  Matmuls run in float32r (full-rate fp32 on the PE).
"""

import numpy as np

try:
    from concourse import bass, mybir, tile
    from concourse.bass_utils import run_bass_kernel_spmd
except ImportError:  # pragma: no cover
    import sys

    sys.path.insert(0, "/opt/trn_rl_repo")
    from concourse import bass, mybir, tile
    from concourse.bass_utils import run_bass_kernel_spmd

# The kernel-tail Drain emitted by TileContext waits on every proc at once
# (12+ sem waits) which overflows the CTRL template's sync-wait slots in this
# walrus build.  Split it: one drain per proc, each carrying a single wait.
from concourse.vector_clock import ScopedClock as _ScopedClock
from concourse.vector_clock import VectorClock as _VectorClock


def _split_drain_and_barrier(self, tick_clock, wait_clock):
    gv = tick_clock.global_clock
    n = len(gv)
    for j in range(n):
        t = gv[j]
        if t <= 0:
            continue
        sub = _VectorClock([t if i == j else 0 for i in range(n)])
        d = self.nc.sync.drain()
        wait_clock.add_sem_waits(d.ins, _ScopedClock({None: sub}))
    self.nc.all_engine_barrier()
    assert self.sems is not None
    popped = self.nc._tile_sem_poison_stack.pop()
    assert popped is self._sem_poison
    self.nc.clear_and_free_semaphores(list(self.sems.allocated().values()))
    self.nc.all_engine_barrier()


tile.TileContext._drain_and_barrier = _split_drain_and_barrier

B, NP, NS, D = 8, 512, 512, 512
PT = 128  # partition tile
KC = D // PT  # 4 chunks
F32 = mybir.dt.float32
F32R = mybir.dt.float32r
I32 = mybir.dt.int32
AX = mybir.AxisListType.X
ALU = mybir.AluOpType
AF = mybir.ActivationFunctionType
SCALE_QK = 0.125  # per-side sqrt of 1/64


def _emit(nc, io):
    """Emit the single-core program (SPMD: every core runs this on its batch)."""
    from contextlib import ExitStack

    with tile.TileContext(nc) as tc, ExitStack() as ctx:
        pers = ctx.enter_context(tc.tile_pool(name="pers", bufs=1))
        psb = ctx.enter_context(tc.tile_pool(name="psb", bufs=4, space="PSUM"))
        psr = ctx.enter_context(tc.tile_pool(name="psr", bufs=2, space="PSUM"))
        psc = ctx.enter_context(tc.tile_pool(name="psc", bufs=2, space="PSUM"))

        def big(name, dtype=F32):
            return pers.tile([PT, KC * 512], dtype, name=name, tag=name)

        def small(name, shape, dtype=F32):
            return pers.tile(shape, dtype, name=name, tag=name)

        # ---- persistent SBUF tiles -------------------------------------
        pT = big("pT", F32R)    # P^T  [d_in, p]   chunk k at cols 512k..
        sT = big("sT", F32R)    # S^T  [d_in, s]
        wq = big("wq", F32R)    # Wq   [d_in, d_out]
        wk = big("wk", F32R)
        wv = big("wv", F32R)
        wo = big("wo", F32R)
        qT = big("qT", F32R)    # Q^T/8 [d_out, p]
        kT = big("kT", F32R)    # K^T/8 [d_out, s]
        vv = big("vv", F32R)    # V0 = S@Wv  [s, d]
        mask = big("maskb", I32)
        sbias = big("sbiasb")
        cb = big("cbb")         # spatial_bias + (mask-1)*1e9
        sout = big("soutb")     # final scores (output 0)
        eef = big("eefb")       # exp(sout - rowmax), f32 from ACT
        ee = big("eeb", F32R)   # DVE-rounded copy of eef

        bqr = small("bqr", [1, 512], F32R)
        bkr = small("bkr", [1, 512], F32R)
        wcp = [small(f"wcp{m}", [PT, 1], F32R) for m in range(KC)]
        wcs = [small(f"wcs{m}", [PT, 1], F32R) for m in range(KC)]
        bvr = small("bvr", [1, 512])
        bvrr = small("bvrr", [1, 512], F32R)
        bor = small("bor", [1, 512])
        wvalr = small("wvalr", [1, 512])
        bvalr = small("bvalr", [1, 1])
        bcr = small("bcr", [1, 512], F32R)

        negmx = [small(f"negmx{m}", [PT, 1]) for m in range(KC)]
        esum = [small(f"esum{m}", [PT, 1]) for m in range(KC)]
        ers = [small(f"ers{m}", [PT, 1], F32R) for m in range(KC)]
        cprow = small("cprow", [1, 512], F32R)
        csrow = small("csrow", [1, 512], F32R)
        ones1_128f = small("ones1_128f", [1, 128])
        ones1_512f = small("ones1_512f", [1, 512])
        onesf = small("onesf", [1, 1])
        c512f = small("c512f", [1, 1])
        ones1_128 = small("ones1_128", [1, 128], F32R)
        ones1_512 = small("ones1_512", [1, 512], F32R)
        ones11 = small("ones11", [1, 1], F32R)
        c512r = small("c512r", [1, 1], F32R)
        crow = small("crow", [1, 512], F32R)
        ccol = [small(f"ccol{m}", [PT, 1], F32R) for m in range(KC)]
        trow2 = small("trow2", [1, 512], F32R)
        tcol = [small(f"tcol{m}", [PT, 1], F32R) for m in range(KC)]
        morow = small("morow", [1, 512])
        prodr = small("prodr", [1, 512])
        vsum = small("vsum", [1, 1])
        val = small("val", [1, 1])

        def blk(t, k):
            return t[:, 512 * k : 512 * (k + 1)]

        def col(t, k, m):
            return t[:, 512 * k + 128 * m : 512 * k + 128 * m + 128]

        def funnel(ap):
            # DVE self-copy: collapses the (multi-queue) DMA deps of `ap`
            # onto the DVE semaphore so consuming matmuls need one wait.
            nc.vector.tensor_copy(ap, ap)

        # ---- DMA inputs + DVE funnels (in consumption order) -----------
        for k in range(KC):
            rows = slice(PT * k, PT * (k + 1))
            nc.sync.dma_start(out=blk(pT, k), in_=io["pT"][rows, :])
            nc.sync.dma_start(out=blk(wq, k), in_=io["Wq"][rows, :])
        nc.sync.dma_start(out=bqr[:], in_=io["bqr"][:])
        nc.sync.dma_start(out=bkr[:], in_=io["bkr"][:])
        for m in range(KC):
            nc.sync.dma_start(out=wcp[m][:], in_=io["wcp"][m])
            nc.sync.dma_start(out=wcs[m][:], in_=io["wcs"][m])
        funnel(bqr[:])
        funnel(bkr[:])
        for k in range(KC):
            funnel(blk(pT, k))
            funnel(blk(wq, k))
        for k in range(KC):
            rows = slice(PT * k, PT * (k + 1))
            nc.sync.dma_start(out=blk(sT, k), in_=io["sT"][rows, :])
            nc.sync.dma_start(out=blk(wk, k), in_=io["Wk"][rows, :])
        for k in range(KC):
            funnel(blk(sT, k))
            funnel(blk(wk, k))
        for m in range(KC):
            funnel(wcp[m][:])
            funnel(wcs[m][:])
        for k in range(KC):
            rows = slice(PT * k, PT * (k + 1))
            nc.sync.dma_start(out=blk(wv, k), in_=io["Wv"][rows, :])
        for k in range(KC):
            funnel(blk(wv, k))
        for k in range(KC):
            rows = slice(PT * k, PT * (k + 1))
            nc.sync.dma_start(out=blk(mask, k), in_=io["mask"][rows, :])
            nc.sync.dma_start(out=blk(sbias, k), in_=io["spatial_bias"][rows, :])
        for k in range(KC):
            rows = slice(PT * k, PT * (k + 1))
            nc.sync.dma_start(out=blk(wo, k), in_=io["Wo"][rows, :])
        for k in range(KC):
            funnel(blk(wo, k))
        nc.sync.dma_start(out=bvr[:], in_=io["bvr"][:])
        nc.sync.dma_start(out=bor[:], in_=io["bor"][:])
        nc.sync.dma_start(out=wvalr[:], in_=io["wvalr"][:])
        nc.sync.dma_start(out=bvalr[:], in_=io["bvalr"][:])
        nc.sync.dma_start(out=bcr[:], in_=io["bcr"][:])
        nc.vector.tensor_copy(bvrr[:], bvr[:])
        funnel(bor[:])
        funnel(wvalr[:])
        funnel(bvalr[:])
        funnel(bcr[:])

        # ---- constants (DVE-produced so matmuls wait on DVE only) ------
        nc.vector.memset(onesf[:], 1.0)
        nc.vector.memset(ones1_128f[:], 1.0)
        nc.vector.memset(ones1_512f[:], 1.0)
        nc.vector.memset(c512f[:], float(NP))
        nc.vector.tensor_copy(ones11[:], onesf[:])
        nc.vector.tensor_copy(ones1_128[:], ones1_128f[:])
        nc.vector.tensor_copy(ones1_512[:], ones1_512f[:])
        nc.vector.tensor_copy(c512r[:], c512f[:])

        # ---- combined additive bias on GPSIMD: cb = sbias + (mask-1)*1e9
        for m in range(KC):
            mf = blk(cb, m)
            # self-funnel sbias onto the Pool clock so the add has one wait
            nc.gpsimd.tensor_copy(blk(sbias, m), blk(sbias, m))
            nc.gpsimd.tensor_copy(mf, blk(mask, m))  # int32 -> f32 cast
            nc.gpsimd.tensor_scalar(mf, mf, 1e9, -1e9, ALU.mult, ALU.add)
            nc.gpsimd.tensor_tensor(mf, mf, blk(sbias, m), ALU.add)
            funnel(mf)  # collapse the GPSIMD dep onto the DVE clock

        # ---- Q^T and K^T ------------------------------------------------
        for m in range(KC):
            psq = psb.tile([PT, 512], F32, name=f"psq{m}", tag="bigp")
            for k in range(KC):
                nc.tensor.matmul(
                    psq[:], col(wq, k, m), blk(pT, k),
                    start=(k == 0), stop=False,
                )
            nc.tensor.matmul(  # + bq[chunk] (x) ones
                psq[:], bqr[0:1, 128 * m : 128 * (m + 1)], ones1_512[:],
                start=False, stop=True,
            )
            # qT = psq*0.125   (DVE, writes f32r; imm-only -> 1 wait)
            with nc.allow_low_precision(reason="f32r rounding for PE operand"):
                nc.vector.tensor_scalar(
                    blk(qT, m), psq[:], SCALE_QK, None, ALU.mult
                )
        for m in range(KC):
            psk = psb.tile([PT, 512], F32, name=f"psk{m}", tag="bigp")
            for k in range(KC):
                nc.tensor.matmul(
                    psk[:], col(wk, k, m), blk(sT, k),
                    start=(k == 0), stop=False,
                )
            nc.tensor.matmul(
                psk[:], bkr[0:1, 128 * m : 128 * (m + 1)], ones1_512[:],
                start=False, stop=True,
            )
            with nc.allow_low_precision(reason="f32r rounding for PE operand"):
                nc.vector.tensor_scalar(
                    blk(kT, m), psk[:], SCALE_QK, None, ALU.mult
                )

        # ---- compat row GEMVs ------------------------------------------
        cpr = psr.tile([1, 512], F32, name="cpr", tag="rowp")
        for k in range(KC):
            nc.tensor.matmul(
                cpr[:], wcp[k][:], blk(pT, k),
                start=(k == 0), stop=(k == KC - 1),
            )
        csr = psr.tile([1, 512], F32, name="csr", tag="rowp")
        for k in range(KC):
            nc.tensor.matmul(
                csr[:], wcs[k][:], blk(sT, k),
                start=(k == 0), stop=(k == KC - 1),
            )
        nc.vector.tensor_copy(cprow[:], cpr[:])
        nc.vector.tensor_copy(csrow[:], csr[:])

        # ---- V ----------------------------------------------------------
        for m in range(KC):
            psv = psb.tile([PT, 512], F32, name=f"psv{m}", tag="bigp")
            for k in range(KC):
                nc.tensor.matmul(
                    psv[:], col(sT, k, m), blk(wv, k),
                    start=(k == 0), stop=(k == KC - 1),
                )
            nc.vector.tensor_copy(blk(vv, m), psv[:])

        # ---- scores + softmax stats ------------------------------------
        for m in range(KC):
            pss = psb.tile([PT, 512], F32, name=f"pss{m}", tag="bigp")
            for k in range(KC):
                nc.tensor.matmul(
                    pss[:], col(qT, k, m), blk(kT, k),
                    start=(k == 0), stop=(k == KC - 1),
                )
            so = blk(sout, m)
            nc.vector.tensor_copy(so, pss[:])  # 1 wait: PE
            nc.vector.tensor_add(so, so, blk(cb, m))  # DVE-internal
            if m == 1:  # blocks 0+1 on one SWDGE proc (its only transfer)
                nc.gpsimd.dma_start(
                    out=io["out_scores"].rearrange("(m p) d -> p m d", m=KC)[
                        :, 0:2, :
                    ],
                    in_=sout[:, 0:1024].rearrange("p (m d) -> p m d", m=2),
                )
            elif m >= 2:
                nc.gpsimd.dma_start(
                    out=io["out_scores"][PT * m : PT * (m + 1), :], in_=so
                )
            nc.vector.tensor_reduce(
                negmx[m][:], so, axis=AX, op=ALU.max, negate=True
            )
            nc.scalar.activation(
                blk(eef, m), so, AF.Exp, bias=negmx[m][:], scale=1.0
            )
            nc.vector.tensor_copy(blk(ee, m), blk(eef, m))  # f32r round
            nc.vector.tensor_reduce(esum[m][:], blk(ee, m), axis=AX, op=ALU.add)
            with nc.allow_low_precision(reason="f32r rounding for PE operand"):
                nc.vector.reciprocal(ers[m][:], esum[m][:])

        # ---- compatibility tiles ---------------------------------------
        for m in range(KC):
            psco = psb.tile([PT, 512], F32, name=f"psco{m}", tag="bigp")
            nc.tensor.matmul(
                psco[:], cprow[0:1, 128 * m : 128 * (m + 1)], ones1_512[:],
                start=True, stop=False,
            )
            nc.tensor.matmul(
                psco[:], ones1_128[:], csrow[:], start=False, stop=False,
            )
            nc.tensor.matmul(  # + bc everywhere
                psco[:], bcr[0:1, 0:128], ones1_512[:], start=False, stop=True,
            )
            cot = pers.tile([PT, 512], F32, name=f"cot{m}", tag=f"cot{m}")
            nc.vector.tensor_copy(cot[:], psco[:])
            nc.gpsimd.dma_start(
                out=io["out_compat"][PT * m : PT * (m + 1), :], in_=cot[:]
            )

        # ---- colsum_probs = sum_p probs[p, :]  (row vector) ------------
        crp = psr.tile([1, 512], F32, name="crp", tag="rowp")
        for m in range(KC):
            nc.tensor.matmul(
                crp[:], ers[m][:], blk(ee, m),
                start=(m == 0), stop=(m == KC - 1),
            )
        nc.vector.tensor_copy(crow[:], crp[:])
        # transpose row -> 4 column chunks via K=1 matmuls with ones[1,1]
        for m in range(KC):
            pst = psc.tile([PT, 1], F32, name=f"pst{m}", tag="colp")
            nc.tensor.matmul(
                pst[:], crow[0:1, 128 * m : 128 * (m + 1)].bitcast(F32),
                onesf[:], start=True, stop=True,
            )
            nc.vector.tensor_copy(ccol[m][:], pst[:])

        # ---- a0 = colsum_probs @ V0 + Np*bv ; t = a0/Np ----------------
        pa0 = psr.tile([1, 512], F32, name="pa0", tag="rowp")
        for k in range(KC):
            nc.tensor.matmul(
                pa0[:], ccol[k][:], blk(vv, k),
                start=(k == 0), stop=False,
            )
        nc.tensor.matmul(pa0[:], c512r[:], bvrr[:], start=False, stop=True)
        with nc.allow_low_precision(reason="f32r rounding for PE operand"):
            nc.vector.tensor_scalar(
                trow2[:], pa0[:], 1.0 / NP, None, ALU.mult
            )
        for m in range(KC):
            pst2 = psc.tile([PT, 1], F32, name=f"pst2{m}", tag="colp")
            nc.tensor.matmul(
                pst2[:], trow2[0:1, 128 * m : 128 * (m + 1)].bitcast(F32),
                onesf[:], start=True, stop=True,
            )
            nc.vector.tensor_copy(tcol[m][:], pst2[:])

        # ---- mean_out = t @ Wo + bo ; value = mean_out . Wval + bval ---
        pmo = psr.tile([1, 512], F32, name="pmo", tag="rowp")
        for k in range(KC):
            nc.tensor.matmul(
                pmo[:], tcol[k][:], blk(wo, k),
                start=(k == 0), stop=(k == KC - 1),
            )
        nc.vector.tensor_copy(morow[:], pmo[:])
        nc.vector.tensor_add(morow[:], morow[:], bor[:])
        nc.vector.tensor_mul(prodr[:], morow[:], wvalr[:])
        nc.vector.tensor_reduce(vsum[:], prodr[:], axis=AX, op=ALU.add)
        nc.vector.tensor_add(val[:], vsum[:], bvalr[:])
        nc.gpsimd.dma_start(out=io["out_value"][:], in_=val[:])


def build():
    nc = bass.Bass("TRN2", target_bir_lowering=False, debug=False)
    io = {}
    io["pT"] = nc.declare_dram_parameter("pT", [D, NP], F32R, isOutput=False).ap()
    io["sT"] = nc.declare_dram_parameter("sT", [D, NS], F32R, isOutput=False).ap()
    io["mask"] = nc.declare_dram_parameter("mask", [NP, NS], I32, isOutput=False).ap()
    io["spatial_bias"] = nc.declare_dram_parameter(
        "spatial_bias", [NP, NS], F32, isOutput=False
    ).ap()
    for w in ("Wq", "Wk", "Wv", "Wo"):
        io[w] = nc.declare_dram_parameter(w, [D, D], F32R, isOutput=False).ap()
    io["bqr"] = nc.declare_dram_parameter("bqr", [1, 512], F32R, isOutput=False).ap()
    io["bkr"] = nc.declare_dram_parameter("bkr", [1, 512], F32R, isOutput=False).ap()
    io["wcp"] = nc.declare_dram_parameter("wcp", [KC, PT, 1], F32R, isOutput=False).ap()
    io["wcs"] = nc.declare_dram_parameter("wcs", [KC, PT, 1], F32R, isOutput=False).ap()
    io["bvr"] = nc.declare_dram_parameter("bvr", [1, 512], F32, isOutput=False).ap()
    io["bor"] = nc.declare_dram_parameter("bor", [1, 512], F32, isOutput=False).ap()
    io["wvalr"] = nc.declare_dram_parameter("wvalr", [1, 512], F32, isOutput=False).ap()
    io["bvalr"] = nc.declare_dram_parameter("bvalr", [1, 1], F32, isOutput=False).ap()
    io["bcr"] = nc.declare_dram_parameter("bcr", [1, 512], F32R, isOutput=False).ap()
    io["out_scores"] = nc.declare_dram_parameter(
        "out_scores", [NP, NS], F32, isOutput=True
    ).ap()
    io["out_value"] = nc.declare_dram_parameter(
        "out_value", [1, 1], F32, isOutput=True
    ).ap()
    io["out_compat"] = nc.declare_dram_parameter(
        "out_compat", [NP, NS], F32, isOutput=True
    ).ap()
    _emit(nc, io)
    return nc


_NC = None


def _get_nc():
    global _NC
    if _NC is None:
        _NC = build()
    return _NC


def make_in_maps(inputs):
    inp = {k: np.asarray(v) for k, v in inputs.items()}
    Wc = inp["Wc"].astype(np.float32)
    shared = {
        "Wq": np.ascontiguousarray(inp["Wq"], np.float32),
        "Wk": np.ascontiguousarray(inp["Wk"], np.float32),
        "Wv": np.ascontiguousarray(inp["Wv"], np.float32),
        "Wo": np.ascontiguousarray(inp["Wo"], np.float32),
        "bqr": inp["bq"].astype(np.float32).reshape(1, 512),
        "bkr": inp["bk"].astype(np.float32).reshape(1, 512),
        "wcp": Wc[:D].reshape(KC, PT, 1),
        "wcs": Wc[D:].reshape(KC, PT, 1),
        "bvr": inp["bv"].astype(np.float32).reshape(1, 512),
        "bor": inp["bo"].astype(np.float32).reshape(1, 512),
        "wvalr": inp["Wval"].astype(np.float32).reshape(1, 512),
        "bvalr": inp["bval"].astype(np.float32).reshape(1, 1),
        "bcr": np.full((1, 512), float(inp["bc"]), np.float32),
    }
    in_maps = []
    for b in range(B):
        m = dict(shared)
        m["pT"] = np.ascontiguousarray(inp["piece_embeds"][b].T.astype(np.float32))
        m["sT"] = np.ascontiguousarray(inp["slot_embeds"][b].T.astype(np.float32))
        m["mask"] = np.ascontiguousarray(inp["mask"][b].astype(np.int32))
        m["spatial_bias"] = np.ascontiguousarray(
            inp["spatial_bias"][b].astype(np.float32)
        )
        in_maps.append(m)
    return in_maps


def _assemble(results):
    scores = np.stack([results[b]["out_scores"] for b in range(B)]).astype(np.float32)
    value = np.stack(
        [results[b]["out_value"].reshape(1) for b in range(B)]
    ).astype(np.float32)
    compat = np.stack([results[b]["out_compat"] for b in range(B)]).astype(np.float32)
    return scores, value, compat


def kernel(**inputs):
    nc = _get_nc()
    in_maps = make_in_maps(inputs)
    res = run_bass_kernel_spmd(nc, in_maps, core_ids=list(range(B)))
    return _assemble(res.results)


def _install_ntff_hook():
    """Recreate antenv.axon_hooks + register the ctypes NTFF profile hook
    (the agent image's antenv lacks axon_hooks; trn_boot degrades silently)."""
    import contextlib
    import ctypes
    import sys
    import types

    if "antenv.axon_hooks" not in sys.modules:
        mod = types.ModuleType("antenv.axon_hooks")
        mod._hook = None

        def set_axon_ntff_profile_hook(h):
            mod._hook = h

        def get_axon_ntff_profile_hook():
            return mod._hook

        mod.set_axon_ntff_profile_hook = set_axon_ntff_profile_hook
        mod.get_axon_ntff_profile_hook = get_axon_ntff_profile_hook
        sys.modules["antenv.axon_hooks"] = mod
        import antenv

        antenv.axon_hooks = mod
    mod = sys.modules["antenv.axon_hooks"]
    if mod._hook is not None:
        return

    so_path = "/opt/axon/libaxon_pjrt.so"
    lib = ctypes.CDLL(so_path)
    if not hasattr(lib, "axon_start_nrt_profile"):
        return
    lib.axon_start_nrt_profile.argtypes = [
        ctypes.POINTER(ctypes.c_int64),
        ctypes.c_size_t,
    ]
    lib.axon_start_nrt_profile.restype = ctypes.c_int64
    lib.axon_stop_nrt_profile.argtypes = [ctypes.c_char_p]
    lib.axon_stop_nrt_profile.restype = ctypes.c_int64

    @contextlib.contextmanager
    def _hook(output_dir, device_ids):
        import jax

        jax.devices()
        if device_ids:
            ids = (ctypes.c_int64 * len(device_ids))(*device_ids)
            rc = lib.axon_start_nrt_profile(ids, len(device_ids))
        else:
            rc = lib.axon_start_nrt_profile(None, 0)
        if rc != 0:
            raise RuntimeError(f"axon_start_nrt_profile rc={rc}")
        try:
            yield
        finally:
            n = lib.axon_stop_nrt_profile(str(output_dir).encode())
            print(f"profile: {n} file(s) written to {output_dir}")

    mod.set_axon_ntff_profile_hook(_hook)


def kernel_timed(**inputs):
    """Like kernel() but with NTFF profiling; returns (outputs, exec_time_ns)."""
    from concourse import bass_utils as _bu

    _install_ntff_hook()
    _bu.upload_artifacts = lambda tmpdir: str(tmpdir)  # no bucket in this env
    nc = _get_nc()
    in_maps = make_in_maps(inputs)
    res = run_bass_kernel_spmd(nc, in_maps, core_ids=list(range(B)), trace=True)
    return _assemble(res.results), res.exec_time_ns
